# revision 1
# baseline (speedup 1.0000x reference)
"""Sliding-window causal self-attention (GQA + RoPE + QK-RMSnorm + gated
value-embedding) for Trainium2, SPMD over 8 NeuronCores.

Sharding: core c = (b, g) with b = c // 4 (batch), g = c % 4 (kv head group).
Each core computes its 4 query heads / 1 kv head for its batch and produces a
partial output projection [T, E] (bf16); the host sums the 4 partials per
batch (tensor-parallel all-reduce done host-side).

v4 layout/schedule:
 - phases software-pipelined per 512-token chunk s:
   D2(s) -> E(s) heads interleaved with F(s-1) units -> B(s+1) -> C(s+1)
   -> D1(s+1), so the cross-engine rmsnorm chain and the output projection
   hide under attention.
 - one PSUM ring: all misc matmul outputs pack pairwise into [128,1024]
   tiles sharing a single 3-deep pool (12KB) + double-buffered pv (4KB),
   which buys attention a 2-tile score pipeline lead.
 - attention score tiles column-trimmed and compactly packed: diagonal
   chunk d pairs with far chunk d-1024 in one [128, 640] region, one exp
   per tile.  Edge triangles masked by accumulating -240 bias tiles into
   score PSUM via identity matmuls (exp(-30) == 0): no DVE mask work.
 - rmsnorm rstd = exp(-0.5*ln(mean+eps)) on Act; sigmoid gates hoisted to
   kernel start and a hand-pinned natural_log_exp table load keep the
   whole kernel at 3 act-table loads.
 - denominator via ones-column in v (row 64), reciprocal straight to bf16,
   broadcast by a K=1 matmul, final normalize one DVE mul on two PSUM
   operands.  Output written bf16 (host accumulates in fp32).
"""

import numpy as np
import ml_dtypes

import concourse.bass as bass
import concourse.mybir as mybir
import concourse.tile as tile
from concourse import bacc
from concourse.bass_utils import run_bass_kernel_spmd
from concourse.instruction_name_ordered_set import InstructionNameOrderedSet

BF16 = mybir.dt.bfloat16
F32 = mybir.dt.float32
BF = ml_dtypes.bfloat16

T, E, H, HKV, D = 2048, 1024, 16, 4, 64
WIN = 1024
NQH = 4            # q heads per core
NQ = NQH * D       # 256 q dims per core
EPS = 1.1920929e-07
NEG = -240.0       # pre-scale mask bias: exp(0.125 * -240) = exp(-30) ~ 0


def _tiles(qc):
    """Score-tile packing for q rows [512qc, 512qc+512).

    Returns a list of (W, chunks); chunk = (ka, place0, w, qcol0, tri) with
    tri = None | (mask, tri_place0); mask in {'b1','b2'}.  Scores for key
    chunk ka land in s[:, place0:place0+w] covering q cols [qcol0, qcol0+w).
    """
    t0 = 512 * qc
    kas = list(range(max(0, 4 * qc - 8), 4 * qc + 4))
    interior = [ka for ka in kas if -512 <= 128 * ka - t0 <= -128]
    diag = [ka for ka in kas if 128 * ka - t0 >= 0]
    far = [ka for ka in kas if 128 * ka - t0 <= -640]
    tiles = []
    for i in range(0, len(interior), 2):
        a, b = interior[i], interior[i + 1]
        tiles.append((1024, [(a, 0, 512, 0, None), (b, 512, 512, 0, None)]))
    if far:
        for dka in diag:
            d = 128 * dka - t0
            fka = dka - 8          # its far partner, offset d - 1024
            wd = 512 - d
            tiles.append((640, [
                (dka, 0, wd, d, ('b1', 0)),
                (fka, wd, d + 128, 0, ('b2', 512)),
            ]))
    elif diag:
        d0, d1, d2, d3 = diag
        tiles.append((896, [
            (d0, 0, 512, 0, ('b1', 0)),
            (d1, 512, 384, 128, ('b1', 512)),
        ]))
        tiles.append((384, [
            (d2, 0, 256, 256, ('b1', 0)),
            (d3, 256, 128, 384, ('b1', 256)),
        ]))
    return tiles


TILES = [_tiles(qc) for qc in range(4)]


def emit_consts(nc, tc, dram, pools):
    """One-time setup: act table, weights, constants, persistent tiles."""
    big, work, psp, psE = pools

    # the whole kernel uses only copy/ln/exp: pin the combined
    # natural_log_exp table once so the auto-inserter doesn't thrash
    nc.scalar.add_instruction(mybir.InstLoadActFuncSet(
        name=nc.get_next_instruction_name(), ins=[], outs=[],
        act_func_set_id=6))

    st = {}
    wqg_all = big.tile([128, 8, 257], BF16, tag="wqg", name="wqg")
    nc.sync.dma_start(wqg_all[:], dram["wqg"].rearrange("(e p) m -> p e m", p=128))
    st["wqg_sb"] = [wqg_all[:, i, :] for i in range(8)]
    wkv_all = big.tile([128, 8, 128], BF16, tag="wkv", name="wkv")
    nc.scalar.dma_start(wkv_all[:], dram["wkv"].rearrange("(e p) m -> p e m", p=128))
    st["wkv_sb"] = [wkv_all[:, i, :] for i in range(8)]
    wp_all = big.tile([128, 2, 1024], BF16, tag="wp", name="wp")
    nc.scalar.dma_start(wp_all[:], dram["wp"].rearrange("(e p) m -> p e m", p=128))
    st["wp_sb"] = [wp_all[:, i, :] for i in range(2)]

    for nm, shape in [("crep", [128, T]), ("srep", [128, T]),
                      ("bw65", [65, 448]), ("psw", [128, 128]),
                      ("ident", [128, 128]), ("b1", [128, 128]),
                      ("b2", [128, 128]), ("bind", [128, 3])]:
        t = big.tile(shape, BF16, tag=nm, name=nm)
        eng = nc.sync if len(st) % 2 == 0 else nc.scalar
        eng.dma_start(t[:], dram[nm][:])
        st[nm] = t

    # double-buffered per-rep inputs (rep r+1 loads overlap rep r compute)
    st["xT"] = [big.tile([128, 8, T], BF16, tag=f"xT{p}", name=f"xT{p}")
                for p in range(2)]
    st["vet2"] = [big.tile([64, T], BF16, tag=f"vet2{p}", name=f"vet2{p}")
                  for p in range(2)]
    st["g2"] = [big.tile([1, T], BF16, tag=f"g2{p}", name=f"g2{p}")
                for p in range(2)]

    # persistent intermediates (cross-rep uses touch disjoint columns)
    st["q_raw"] = [big.tile([128, T], BF16, tag=f"qraw{i}", name=f"qraw{i}")
                   for i in range(2)]
    st["kv_raw"] = big.tile([128, T], BF16, tag="kvraw", name="kvraw")
    st["rot_q"] = [big.tile([128, T], BF16, tag=f"rotq{i}", name=f"rotq{i}")
                   for i in range(2)]
    st["yT"] = [big.tile([128, T], BF16, tag=f"yT{i}", name=f"yT{i}")
                for i in range(2)]
    # k pipeline lives on partitions 64:128 throughout so every
    # tensor-tensor op has equal input start partitions
    st["rot_k"] = big.tile([128, T], BF16, tag="rotk", name="rotk")
    st["sq"] = [big.tile([128, T], BF16, tag=f"sq{i}", name=f"sq{i}")
                for i in range(2)]
    st["sqk"] = big.tile([128, T], BF16, tag="sqk", name="sqk")
    st["rstd_raw"] = big.tile([65, T], F32, tag="rstdraw", name="rstdraw")
    st["rstd"] = big.tile([65, T], BF16, tag="rstd", name="rstd")
    st["kn_dup"] = big.tile([128, T], BF16, tag="kndup", name="kndup")
    st["vt_fin"] = big.tile([80, T], BF16, tag="vtfin", name="vtfin")
    st["v_sb"] = big.tile([128, 16, 80], BF16, tag="vsb", name="vsb")
    st["epsb"] = big.tile([128, 1], F32, tag="epsb", name="epsb")
    nc.gpsimd.memset(st["epsb"][:], EPS)
    nc.gpsimd.memset(st["rstd_raw"][:], 0.0)
    nc.gpsimd.memset(st["vt_fin"][64:80, :], 0.0)
    nc.gpsimd.memset(st["vt_fin"][64:65, :], 1.0)
    return st


def emit_stream(nc, tc, dram, pools, st, n_reps):
    big, work, psp, psE = pools
    AF = mybir.ActivationFunctionType

    def pst(name):
        return psp.tile([128, 1024], F32, tag="ps", name=name)

    wqg_sb, wkv_sb, wp_sb = st["wqg_sb"], st["wkv_sb"], st["wp_sb"]
    crep, srep, bw65, psw = st["crep"], st["srep"], st["bw65"], st["psw"]
    ident, b1, b2, bind = st["ident"], st["b1"], st["b2"], st["bind"]
    epsb = st["epsb"]
    q_raw, kv_raw = st["q_raw"], st["kv_raw"]
    qn = q_raw                   # normalized q overwrites raw (raw dead)
    rot_q, yT, rot_k = st["rot_q"], st["yT"], st["rot_k"]
    sq, sqk = st["sq"], st["sqk"]
    rstd_raw, rstd = st["rstd_raw"], st["rstd"]
    kn_dup, vt_fin, v_sb = st["kn_dup"], st["vt_fin"], st["v_sb"]
    xTr = dram["xT"].rearrange("(e p) t -> p e t", p=128)

    def xs(p, i):
        return st["xT"][p][:, i, :]

    def emit_inputs(p):
        xT_all = st["xT"][p]
        order = [(c, 0) for c in range(4)] + [(c, e) for c in range(4)
                                              for e in range(1, 8)]
        for n, (c, e) in enumerate(order):
            eng = nc.sync if n % 2 == 0 else nc.scalar
            cs = slice(512 * c, 512 * (c + 1))
            eng.dma_start(xT_all[:, e:e + 1, cs], xTr[:, e:e + 1, cs])
        nc.scalar.dma_start(st["vet2"][p][:], dram["vet2"][:])

    def emit_gates(p):
        # gates: 2*sigmoid(z) = 2/(1+exp(-z)), exp-table only
        g2 = st["g2"][p]
        ge = work.tile([1, T], BF16, tag="ge", name="ge", bufs=1)
        for cp in range(2):
            gps = pst("gps")
            for j in range(2):
                c = 2 * cp + j
                cs = slice(512 * c, 512 * (c + 1))
                nc.tensor.matmul(gps[0:1, 512 * j:512 * (j + 1)],
                                 wqg_sb[0][:, 256:257], xs(p, 0)[:, cs],
                                 start=True, stop=True)
                nc.scalar.activation(ge[:, cs], gps[0:1, 512 * j:512 * (j + 1)],
                                     AF.Exp, scale=-1.0)
        gp1 = work.tile([1, T], F32, tag="gp1", name="gp1", bufs=1)
        nc.vector.tensor_scalar(gp1[:], ge[:], 1.0, 0.0,
                                mybir.AluOpType.add, mybir.AluOpType.add)
        with nc.allow_low_precision("gate denominator recip in bf16"):
            nc.vector.reciprocal(g2[:], gp1[:])

    def emit_B(tc4, p):
        cs = slice(512 * tc4, 512 * (tc4 + 1))
        g2 = st["g2"][p]
        vet2 = st["vet2"][p]
        qps = pst("qps")
        for i in range(2):
            h = slice(512 * i, 512 * (i + 1))
            for e in range(8):
                nc.tensor.matmul(qps[:, h], wqg_sb[e][:, 128 * i:128 * (i + 1)],
                                 xs(p, e)[:, cs], start=(e == 0), stop=(e == 7))
            nc.scalar.copy(q_raw[i][:, cs], qps[:, h])
        kvb = pst("kvb")
        for e in range(8):
            nc.tensor.matmul(kvb[:, 0:512], wkv_sb[e][:], xs(p, e)[:, cs],
                             start=(e == 0), stop=(e == 7))
        nc.scalar.copy(kv_raw[:, cs], kvb[:, 0:512])
        # V assembly: v + 2*sigmoid(gate)*ve, ones row for the denominator.
        # v sits in rows 0:64 so every tensor-tensor input starts at
        # partition 0 (BIR verifier: inputs must share a start partition).
        nc.tensor.matmul(kvb[0:64, 512:1024], bw65[0:1, 384:448], g2[:, cs],
                         start=True, stop=True)
        vtmp = work.tile([64, 512], BF16, tag="vtmp", name="vtmp", bufs=2)
        nc.vector.tensor_mul(vtmp[:], kvb[0:64, 512:1024], vet2[:, cs])
        nc.vector.tensor_add(vt_fin[0:64, cs], kv_raw[0:64, cs], vtmp[:])
        for kk in range(4):
            ka = 4 * tc4 + kk
            nc.sync.dma_start_transpose(v_sb[:, ka, 0:80],
                                        vt_fin[:, 128 * ka:128 * (ka + 1)])

    def emit_C(tc4):
        cs = slice(512 * tc4, 512 * (tc4 + 1))
        sw01 = pst("sw01")
        swkD = pst("swkD")
        # k lives on partitions 64:128 end-to-end; crep/srep/psw rows 64:128
        # repeat rows 0:64, so all input start partitions stay equal.
        srcs = [(q_raw[0][:, cs], rot_q[0][:, cs], sw01[:, 0:512],
                 crep[:, cs], srep[:, cs]),
                (q_raw[1][:, cs], rot_q[1][:, cs], sw01[:, 512:1024],
                 crep[:, cs], srep[:, cs]),
                (kv_raw[64:128, cs], rot_k[64:128, cs], swkD[64:128, 0:512],
                 crep[64:128, cs], srep[64:128, cs])]
        for i, (src, rot, swp, cr, sr) in enumerate(srcs):
            pp, b0 = (128, 0) if i < 2 else (64, 64)
            nc.tensor.matmul(swp, psw[b0:b0 + pp, b0:b0 + pp], src,
                             start=True, stop=True)
            t2 = work.tile([128, 512], BF16, tag="t2", name="t2", bufs=3)
            nc.vector.tensor_mul(rot, src, cr)
            nc.vector.tensor_mul(t2[b0:b0 + pp, :], swp, sr)
            nc.vector.tensor_add(rot, rot, t2[b0:b0 + pp, :])
        # sum-of-squares sources
        nc.vector.tensor_mul(sq[0][:, cs], rot_q[0][:, cs], rot_q[0][:, cs])
        nc.vector.tensor_mul(sq[1][:, cs], rot_q[1][:, cs], rot_q[1][:, cs])
        nc.vector.tensor_mul(sqk[64:128, cs], rot_k[64:128, cs],
                             rot_k[64:128, cs])
        return swkD

    def emit_D1(tc4, swkD):
        # rstd = exp(-0.5*ln(mean+eps)): sumsq matmuls + Act ln/exp
        cs = slice(512 * tc4, 512 * (tc4 + 1))
        psD = swkD[:, 512:1024]
        nc.tensor.matmul(psD[0:2, :], bind[:, 0:2], sq[0][:, cs],
                         start=True, stop=True)
        nc.tensor.matmul(psD[32:34, :], bind[:, 0:2], sq[1][:, cs],
                         start=True, stop=True)
        nc.tensor.matmul(psD[64:65, :], bind[64:128, 2:3], sqk[64:128, cs],
                         start=True, stop=True)
        for (r0, nb) in ((0, 2), (32, 2), (64, 1)):
            nc.scalar.activation(rstd_raw[r0:r0 + nb, cs], psD[r0:r0 + nb, :],
                                 AF.Ln, scale=1.0 / 64.0,
                                 bias=epsb[r0:r0 + nb, :])
        nc.scalar.activation(rstd[:, cs], rstd_raw[:, cs], AF.Exp, scale=-0.5)

    def emit_D2(tc4):
        # broadcast rstd and normalize q/k
        cs = slice(512 * tc4, 512 * (tc4 + 1))
        bps01 = pst("bps01")
        for ti in range(2):
            h = slice(512 * ti, 512 * (ti + 1))
            nc.tensor.matmul(bps01[:, h], bw65[:, 128 * ti:128 * (ti + 1)],
                             rstd[:, cs], start=True, stop=True)
            nc.vector.tensor_mul(qn[ti][:, cs], rot_q[ti][:, cs], bps01[:, h])
        bpsk = pst("bpsk")
        nc.tensor.matmul(bpsk[64:128, 0:512], bw65[:, 256:320], rstd[:, cs],
                         start=True, stop=True)
        nc.vector.tensor_mul(kn_dup[64:128, cs], rot_k[64:128, cs],
                             bpsk[64:128, 0:512])
        nc.sync.dma_start(kn_dup[0:64, cs], kn_dup[64:128, cs])

    def emit_E_head(qc, h):
        t0 = 512 * qc
        qs = slice(t0, t0 + 512)
        tiles = TILES[qc]
        n = len(tiles)
        ti, base = h // 2, 64 * (h % 2)
        pvt = psE.tile([128, 512], F32, tag="pv", name="pv")
        pv = pvt[0:65, :]
        s_t = [None] * n
        p_t = [None] * n
        nchunks = sum(len(cl) for (_, cl) in tiles)
        state = {"ci": 0}

        def emit_s(i):
            W, chunks = tiles[i]
            s_t[i] = pst("s")
            for (ka, p0, w, q0, tri) in chunks:
                # split matmul outputs at the 512-col PSUM bank boundary
                segs = ([(p0, w)] if p0 >= 512 or p0 + w <= 512
                        else [(p0, 512 - p0), (512, p0 + w - 512)])
                for (sp, sw_) in segs:
                    q1 = t0 + q0 + sp - p0
                    nc.tensor.matmul(
                        s_t[i][:, sp:sp + sw_],
                        kn_dup[base:base + 64, 128 * ka:128 * (ka + 1)],
                        qn[ti][base:base + 64, q1:q1 + sw_],
                        start=True, stop=(tri is None), skip_group_check=True)
                if tri is not None:
                    m, tp = tri
                    nc.tensor.matmul(
                        s_t[i][:, tp:tp + 128], ident[:],
                        b1[:] if m == 'b1' else b2[:],
                        start=False, stop=True, skip_group_check=True)

        def emit_exp(i):
            W, chunks = tiles[i]
            p_t[i] = work.tile([128, 1024], BF16, tag="p", name="p", bufs=5)
            nc.scalar.activation(p_t[i][:, 0:W], s_t[i][:, 0:W], AF.Exp,
                                 scale=0.125)

        def emit_pv(i):
            W, chunks = tiles[i]
            for (ka, p0, w, q0, tri) in chunks:
                nc.tensor.matmul(
                    pv[:, q0:q0 + w], v_sb[:, ka, 0:65],
                    p_t[i][:, p0:p0 + w],
                    start=(state["ci"] == 0), stop=(state["ci"] == nchunks - 1),
                    skip_group_check=True)
                state["ci"] += 1

        # depth-3 software pipeline: PE stays up to 3 score-tiles ahead of PV
        lead = min(3, n)
        expd = 0
        for i in range(lead):
            emit_s(i)
            if i < lead - 1:
                emit_exp(i)
                expd = i + 1
        for i in range(lead, n):
            emit_s(i)
            emit_pv(i - lead)
            emit_exp(expd)
            expd += 1
        while expd < n:
            emit_exp(expd)
            expd += 1
        for i in range(max(0, n - lead), n):
            emit_pv(i)

        recb = work.tile([1, 512], BF16, tag="recb", name="recb", bufs=2)
        with nc.allow_low_precision("softmax denom reciprocal in bf16"):
            nc.vector.reciprocal(recb[:], pv[64:65, :])
        rbp = psE.tile([128, 512], F32, tag="pv", name="rbp")
        nc.tensor.matmul(rbp[0:64, :], bw65[0:1, 384:448], recb[:],
                         start=True, stop=True)
        # DVE may read only one PSUM operand: stage 1/denom in SBUF bf16
        rbs = work.tile([64, 512], BF16, tag="rbs", name="rbs", bufs=2)
        nc.vector.tensor_copy(rbs[:], rbp[0:64, :])
        nc.vector.tensor_mul(yT[ti][base:base + 64, qs],
                             pvt[0:64, :], rbs[:])

    def emit_F_unit(tc4, u):
        tt = 4 * tc4 + u
        ts_ = slice(128 * tt, 128 * (tt + 1))
        fps = pst("fps")
        for nch in range(2):
            for ti in range(2):
                nc.tensor.matmul(fps[:, 512 * nch:512 * (nch + 1)],
                                 yT[ti][:, ts_],
                                 wp_sb[ti][:, 512 * nch:512 * (nch + 1)],
                                 start=(ti == 0), stop=(ti == 1))
        ob = work.tile([128, 1024], BF16, tag="ob", name="ob", bufs=3)
        nc.vector.tensor_copy(ob[:], fps[:])
        deng = nc.sync if tt % 2 == 0 else nc.scalar
        deng.dma_start(dram["out"][ts_, :], ob[:])

    # ---- globally pipelined superstep stream (across reps) ----
    # E(g) runs against B/C/D1(g+1); F(g) lands after D1(g+1) so the PE has
    # filler while Act finishes the ln/exp rstd chain.  Rep r+1's inputs
    # stream in during rep r's second superstep; its gates/projections slot
    # into rep r's attention tail, so rep boundaries stay pipelined.
    G = 4 * n_reps
    emit_inputs(0)
    emit_gates(0)
    emit_B(0, 0)
    swkD = emit_C(0)
    emit_D1(0, swkD)
    pend = []        # F units deferred to fill the D2 -> E normalize gap
    for g in range(G):
        r, s = divmod(g, 4)
        emit_D2(s)
        for (ps_, u) in pend:
            emit_F_unit(ps_, u)
        pend = []
        for h in range(NQH):
            emit_E_head(s, h)
        if s == 1 and r + 1 < n_reps:
            emit_inputs((r + 1) % 2)
        if g + 1 < G:
            r2, s2 = divmod(g + 1, 4)
            p2 = r2 % 2
            if s2 == 0:
                emit_gates(p2)
            emit_B(s2, p2)
            swkD = emit_C(s2)
            emit_D1(s2, swkD)
            emit_F_unit(s, 0)
            emit_F_unit(s, 1)
            pend = [(s, 2), (s, 3)]
        else:
            for u in range(4):
                emit_F_unit(s, u)


def build_nc(n_reps=1):
    nc = bacc.Bacc("TRN2", target_bir_lowering=False, debug=False)
    dram = {
        "xT": nc.dram_tensor("xT", [E, T], BF16, kind="ExternalInput"),
        "wqg": nc.dram_tensor("wqg", [E, 257], BF16, kind="ExternalInput"),
        "wkv": nc.dram_tensor("wkv", [E, 128], BF16, kind="ExternalInput"),
        "wp": nc.dram_tensor("wp", [NQ, E], BF16, kind="ExternalInput"),
        "crep": nc.dram_tensor("crep", [128, T], BF16, kind="ExternalInput"),
        "srep": nc.dram_tensor("srep", [128, T], BF16, kind="ExternalInput"),
        "vet2": nc.dram_tensor("vet2", [64, T], BF16, kind="ExternalInput"),
        "bw65": nc.dram_tensor("bw65", [65, 448], BF16, kind="ExternalInput"),
        "psw": nc.dram_tensor("psw", [128, 128], BF16, kind="ExternalInput"),
        "ident": nc.dram_tensor("ident", [128, 128], BF16, kind="ExternalInput"),
        "b1": nc.dram_tensor("b1", [128, 128], BF16, kind="ExternalInput"),
        "b2": nc.dram_tensor("b2", [128, 128], BF16, kind="ExternalInput"),
        "bind": nc.dram_tensor("bind", [128, 3], BF16, kind="ExternalInput"),
        "out": nc.dram_tensor("out", [T, E], BF16, kind="ExternalOutput"),
    }
    with tile.TileContext(nc) as tc:
        with (
            tc.tile_pool(name="big", bufs=1) as big,
            tc.tile_pool(name="work", bufs=3) as work,
            tc.tile_pool(name="psp", bufs=3, space=bass.MemorySpace.PSUM) as psp,
            tc.tile_pool(name="psE", bufs=2, space=bass.MemorySpace.PSUM) as psE,
        ):
            pools = (big, work, psp, psE)
            st = emit_consts(nc, tc, dram, pools)
            emit_stream(nc, tc, dram, pools, st, n_reps)
    nc.compile()
    return nc


def prep_inputs(x, ve, cos, sin, Wq, Wk, Wv, Wproj, Wgate):
    """Host-side sharding/layout prep -> list of 8 per-core input dicts."""
    x = np.asarray(x, np.float32)
    ve = np.asarray(ve, np.float32)
    cos = np.asarray(cos, np.float32).reshape(T, D // 2)
    sin = np.asarray(sin, np.float32).reshape(T, D // 2)
    Wq = np.asarray(Wq, np.float32)
    Wk = np.asarray(Wk, np.float32)
    Wv = np.asarray(Wv, np.float32)
    Wproj = np.asarray(Wproj, np.float32)
    Wgate = np.asarray(Wgate, np.float32)

    cT = np.ascontiguousarray(cos.T)          # [32, T]
    sT = np.ascontiguousarray(sin.T)
    crep = np.tile(cT, (4, 1)).astype(BF)      # [128, T]
    srep = np.tile(np.concatenate([sT, -sT], 0), (2, 1)).astype(BF)

    bind = np.zeros((128, 3), BF)
    bind[0:64, 0] = 1.0
    bind[64:128, 1] = 1.0
    bind[64:128, 2] = 1.0

    bw65 = np.zeros((65, 448), BF)
    bw65[0, 0:64] = 1.0       # tile0 head0 <- rstd row 0
    bw65[1, 64:128] = 1.0     # tile0 head1 <- rstd row 1
    bw65[32, 128:192] = 1.0   # tile1 head2 <- rstd row 32
    bw65[33, 192:256] = 1.0   # tile1 head3 <- rstd row 33
    bw65[64, 256:320] = 1.0   # k broadcast <- rstd row 64
    bw65[0, 384:448] = 1.0    # ones row (gate / denom broadcasts)

    psw = np.zeros((128, 128), BF)   # RoPE half-swap permutation per 64-block
    for blk in range(2):
        for d in range(32):
            psw[blk * 64 + 32 + d, blk * 64 + d] = 1.0
            psw[blk * 64 + d, blk * 64 + 32 + d] = 1.0

    ident = np.eye(128, dtype=BF)
    j = np.arange(128)[:, None]
    c = np.arange(128)[None, :]
    b1 = np.where(j <= c, 0.0, NEG).astype(BF)   # diag triangle: keep j <= c
    b2 = np.where(j >= c, 0.0, NEG).astype(BF)   # far triangle: keep j >= c

    ins = []
    for core in range(8):
        b, g = core // 4, core % 4
        wgate_pad = np.zeros((E, 1), np.float32)
        wgate_pad[0:32, 0] = Wgate[g]
        wqg = np.concatenate([Wq[NQ * g:NQ * (g + 1)].T, wgate_pad], axis=1)
        # v in cols 0:64, k in cols 64:128 (see emit_B / emit_C)
        wkv = np.concatenate(
            [Wv[D * g:D * (g + 1)].T, Wk[D * g:D * (g + 1)].T], axis=1)
        ins.append({
            "xT": np.ascontiguousarray(x[b].T).astype(BF),
            "wqg": wqg.astype(BF),
            "wkv": wkv.astype(BF),
            "wp": np.ascontiguousarray(Wproj[:, NQ * g:NQ * (g + 1)].T).astype(BF),
            "crep": crep,
            "srep": srep,
            "vet2": np.ascontiguousarray(
                2.0 * ve[b, :, D * g:D * (g + 1)].T).astype(BF),
            "bw65": bw65,
            "psw": psw,
            "ident": ident,
            "b1": b1,
            "b2": b2,
            "bind": bind,
        })
    return ins


_NC_CACHE = {}


def _get_nc(n_reps=1):
    if n_reps not in _NC_CACHE:
        _NC_CACHE[n_reps] = build_nc(n_reps)
    return _NC_CACHE[n_reps]


def kernel(x, ve, cos, sin, Wq, Wk, Wv, Wproj, Wgate, window_size=1024):
    assert int(window_size) == WIN, f"kernel hardcodes window={WIN}"
    ins = prep_inputs(x, ve, cos, sin, Wq, Wk, Wv, Wproj, Wgate)
    nc = _get_nc(1)
    res = run_bass_kernel_spmd(nc, ins, list(range(8)))
    out = np.zeros((2, T, E), np.float32)
    for c in range(8):
        out[c // 4] += res.results[c]["out"].astype(np.float32)
    return out

